# revision 1
# baseline (speedup 1.0000x reference)
"""Trainium2 Bass kernel for MemoryEfficientCrossAttention.

Problem (hardcoded): B=2, Q=2048, K=4096, HIDDEN=1024, HEADS=16, HEAD_DIM=64.
  out = softmax((x_q W_q)(x_k W_k)^T / sqrt(64)) (x_v W_v) W_o

Sharding over 8 NeuronCores: core = g*4 + r
  g in {0,1}: head-group (8 heads -> 512 cols of W_q/W_k/W_v)
  r in {0..3}: 1024-row block of the flattened (B*Q, H) query (batch r//2)
Each core projects q for its rows / k,v for its batch, runs attention for its
(rows x 8 heads), then the (g=0,g=1) pair AllGathers the per-head context
(1 MiB each) and both compute the full W_o product for their row block.

Layout strategy: all matmuls contract over SBUF partitions, so activations are
needed hidden-on-partitions.  fp32 can't use the xbar DMA-transpose, so each
activation row-tile is split into two bf16 planes (hi = bf16(x),
lo = bf16(x - hi)), both planes are xbar-transposed, and the transposed fp32
value is reassembled with one DVE add -- full ~fp32 fidelity at bf16 transpose
cost.  Matmuls run as float32r (full PE rate at N>=256, ~1e-4 rel err).

Scores are computed transposed (S^T[k, q] = k q^T), so exp output is already in
the PV lhsT layout; softmax denominators come from a ones-column appended to V
(PV out row 64), and 1/r is partition-broadcast for the context scale.
"""

import os
import sys
import time

import numpy as np

sys.path.insert(0, "/opt/trn_rl_repo")

import concourse.bass as bass  # noqa: E402
import concourse.mybir as mybir  # noqa: E402
import concourse.tile as tile  # noqa: E402
from concourse import bacc  # noqa: E402
from concourse.bass_utils import run_bass_kernel_spmd  # noqa: E402

F32 = mybir.dt.float32
F32R = mybir.dt.float32r
BF16 = mybir.dt.bfloat16

HID = 1024
HEADS = 16
HD = 64
B = 2
Q = 2048
KL = 4096
NCORE = 8
GC = 512          # head-group cols per core (8 heads)
QR = 1024         # query rows per core
SCALE = HD ** -0.5

_CACHED_NC = None


def _r32(ap):
    return ap


def _build():
    nc = bacc.Bacc("TRN2", target_bir_lowering=False, debug=False,
                   num_devices=NCORE)

    q_rows = nc.dram_tensor("q_rows", [QR, HID], F32, kind="ExternalInput")
    key_b = nc.dram_tensor("key_b", [KL, HID], F32, kind="ExternalInput")
    value_b = nc.dram_tensor("value_b", [KL, HID], F32, kind="ExternalInput")
    wq_s = nc.dram_tensor("wq_s", [HID, GC], F32, kind="ExternalInput")
    wk_s = nc.dram_tensor("wk_s", [HID, GC], F32, kind="ExternalInput")
    wv_s = nc.dram_tensor("wv_s", [HID, GC], F32, kind="ExternalInput")
    w_o = nc.dram_tensor("w_o", [HID, HID], F32, kind="ExternalInput")
    outT = nc.dram_tensor("outT", [HID, QR], F32, kind="ExternalOutput")
    debug = bool(int(os.environ.get("KDEBUG", "0")))
    if debug:
        dbg_qTh = nc.dram_tensor("dbg_qTh", [GC, QR], F32,
                                 kind="ExternalOutput")
        dbg_kTh = nc.dram_tensor("dbg_kTh", [GC, KL], F32,
                                 kind="ExternalOutput")
        dbg_v = nc.dram_tensor("dbg_v", [KL, GC], F32, kind="ExternalOutput")
        dbg_ctx = nc.dram_tensor("dbg_ctx", [GC, QR], F32,
                                 kind="ExternalOutput")
        dbg_gath = nc.dram_tensor("dbg_gath", [HID, QR], F32,
                                  kind="ExternalOutput")

    precise = bool(int(os.environ.get("KPRECISE", "1")))
    NKB = KL // 128           # 32 k-blocks
    NCH = HID // 128          # 8 hidden chunks
    CH_RT = 2                 # row-tiles per transpose chunk (N=256 matmuls)
    CHR = CH_RT * 128         # 256 rows per chunk

    from contextlib import ExitStack

    with tile.TileContext(nc, pool_alloc_mode="queue") as tc:
        with tc.tile_pool(name="dram", bufs=1, space="DRAM") as dram:
            _pst = ExitStack()
            pp = _pst.enter_context(tc.tile_pool(name="persist", bufs=1))
            ctx_own = dram.tile([GC, QR], F32)
            ctx_gath = dram.tile([4, 2 * 128, QR], F32)
            qTh = pp.tile([128, 4, QR], F32R)          # [pair cols, strips, rows]
            v_aug = pp.tile([128, NKB, 8, HD + 1], F32R)
            kTh = pp.tile([128, 4, KL], F32R)

            ones = pp.tile([128, NKB * 8], F32, name="ones")
            nc.vector.memset(ones[:], 1.0)
            nc.vector.tensor_copy(
                v_aug[:, :, :, HD],
                ones[:].rearrange("p (a b) -> p a b", a=NKB))

            # ---------------- projections ----------------
            with (
                tc.tile_pool(name="xstage", bufs=1) as xs,
                tc.tile_pool(name="hilo", bufs=1) as hl,
                tc.tile_pool(name="hiloT", bufs=1) as hlt,
                tc.tile_pool(name="xT", bufs=2) as xts,
                tc.tile_pool(name="wslice", bufs=1) as ws,
                tc.tile_pool(name="pproj", bufs=4, space="PSUM") as pj,
            ):
                def stage_chunk(src, row0):
                    """Return [128, NCH, CHR] f32r transposed chunk of src."""
                    xf = xs.tile([128, CH_RT, HID], F32, tag="xf")
                    for t in range(CH_RT):
                        r0 = row0 + t * 128
                        nc.sync.dma_start(xf[:, t, :], src[r0:r0 + 128, :])
                    hi = hl.tile([128, CH_RT, HID], BF16, tag="hi")
                    nc.vector.tensor_copy(hi[:], xf[:])
                    lo = hl.tile([128, CH_RT, HID], BF16, tag="lo")
                    nc.vector.tensor_sub(lo[:], xf[:], hi[:])
                    hiT = hlt.tile([128, NCH, CHR], BF16, tag="hiT")
                    loT = hlt.tile([128, NCH, CHR], BF16, tag="loT")
                    for t in range(CH_RT):
                        sl = slice(t * 128, (t + 1) * 128)
                        nc.sync.dma_start_transpose(hiT[:, :, sl], hi[:, t, :])
                        nc.sync.dma_start_transpose(loT[:, :, sl], lo[:, t, :])
                    xT = xts.tile([128, NCH, CHR], F32R, tag="xT")
                    nc.vector.tensor_add(xT[:], hiT[:], loT[:])
                    return [xT]

                def load_w(wdram):
                    w = ws.tile([128, NCH, GC], F32R, tag="w")
                    for hc in range(NCH):
                        nc.gpsimd.dma_start(w[:, hc, :],
                                            wdram[hc * 128:(hc + 1) * 128, :])
                    return w

                # q projection -> qTh strips
                w = load_w(wq_s)
                for c in range(QR // CHR):
                    xTh = stage_chunk(q_rows, c * CHR)
                    for s in range(4):
                        ps = pj.tile([128, CHR], F32, tag="pqk")
                        for hc in range(NCH):
                            nc.tensor.matmul(
                                ps[:],
                                _r32(w[:, hc, s * 128:(s + 1) * 128]),
                                _r32(xTh[0][:, hc, :]),
                                start=(hc == 0), stop=(hc == NCH - 1))
                        sl = slice(c * CHR, (c + 1) * CHR)
                        nc.vector.tensor_copy(qTh[:, s, sl], ps[:])

                # k projection -> kTh strips staged to DRAM
                w = load_w(wk_s)
                for c in range(KL // CHR):
                    xTh = stage_chunk(key_b, c * CHR)
                    for s in range(4):
                        ps = pj.tile([128, CHR], F32, tag="pqk")
                        for hc in range(NCH):
                            nc.tensor.matmul(
                                ps[:],
                                _r32(w[:, hc, s * 128:(s + 1) * 128]),
                                _r32(xTh[0][:, hc, :]),
                                start=(hc == 0), stop=(hc == NCH - 1))
                        nc.vector.tensor_copy(
                            kTh[:, s, c * CHR:(c + 1) * CHR], ps[:])

                # v projection -> v_aug natural layout
                w = load_w(wv_s)
                for c in range(KL // CHR):
                    xTh = stage_chunk(value_b, c * CHR)
                    for t in range(CH_RT):
                        ps = pj.tile([128, GC], F32, tag="pv")
                        for hc in range(NCH):
                            nc.tensor.matmul(
                                ps[:],
                                _r32(xTh[0][:, hc,
                                         t * 128:(t + 1) * 128]),
                                _r32(w[:, hc, :]),
                                start=(hc == 0), stop=(hc == NCH - 1))
                        kb = c * CH_RT + t
                        nc.vector.tensor_copy(
                            v_aug[:, kb, :, 0:HD],
                            ps[:].rearrange("p (h d) -> p h d", h=8))

            # ---------------- attention ----------------
            GKB = 3  # k-blocks per score/exp group (3 PSUM banks)
            with (
                tc.tile_pool(name="ctxp", bufs=1) as ctxp,
                tc.tile_pool(name="apool", bufs=3) as apool,
                tc.tile_pool(name="misc", bufs=2) as misc,
                tc.tile_pool(name="pst", bufs=1, space="PSUM") as pst,
                tc.tile_pool(name="pctx", bufs=1, space="PSUM") as pctx,
            ):
                ctxall = ctxp.tile([128, 4, QR], F32)
                for s in range(4):
                    for qb in range(2):
                        qsl = slice(qb * 512, (qb + 1) * 512)
                        ctxs = [pctx.tile([HD + 1, 512], F32, tag=f"ctx{i}",
                                          name=f"ctx{i}_{s}_{qb}")
                                for i in range(2)]
                        for g0 in range(0, NKB, GKB):
                            kbs = list(range(g0, min(g0 + GKB, NKB)))
                            L = len(kbs)
                            sts = [pst.tile([128, GKB * 512], F32,
                                            tag=f"st{i}",
                                            name=f"st{i}_{s}_{qb}_{g0}")
                                   for i in range(2)]
                            for j, kb in enumerate(kbs):
                                ksl = slice(kb * 128, (kb + 1) * 128)
                                jsl = slice(j * 512, (j + 1) * 512)
                                for i in range(2):
                                    psl = slice(i * 64, (i + 1) * 64)
                                    nc.tensor.matmul(
                                        sts[i][:, jsl],
                                        _r32(kTh[psl, s, ksl]),
                                        _r32(qTh[psl, s, qsl]),
                                        start=True, stop=True)
                            As = []
                            for i in range(2):
                                a = apool.tile([128, GKB * 512], F32R,
                                               tag=f"A{i}",
                                               name=f"A{i}_{s}_{qb}_{g0}")
                                nc.scalar.activation(
                                    a[:, 0:L * 512], sts[i][:, 0:L * 512],
                                    mybir.ActivationFunctionType.Exp,
                                    scale=SCALE)
                                As.append(a)
                            for j, kb in enumerate(kbs):
                                jsl = slice(j * 512, (j + 1) * 512)
                                for i in range(2):
                                    nc.tensor.matmul(
                                        ctxs[i][:],
                                        _r32(v_aug[:, kb, 2 * s + i, :]),
                                        _r32(As[i][:, jsl]),
                                        start=(kb == 0), stop=(kb == NKB - 1))
                        for i in range(2):
                            rinv = misc.tile([1, 512], F32, tag="rinv")
                            nc.vector.reciprocal(rinv[:], ctxs[i][HD:HD + 1, :])
                            rb = misc.tile([HD, 512], F32, tag="rb")
                            nc.gpsimd.partition_broadcast(rb[:], rinv[:])
                            nc.vector.tensor_mul(
                                ctxall[i * 64:(i + 1) * 64, s, qsl],
                                ctxs[i][0:HD, :], rb[:])
                    nc.sync.dma_start(ctx_own[s * 128:(s + 1) * 128, :],
                                      ctxall[:, s, :])
                    nc.gpsimd.collective_compute(
                        "AllGather", mybir.AluOpType.bypass,
                        ins=[ctx_own[s * 128:(s + 1) * 128, :]],
                        outs=[ctx_gath[s]],
                        replica_groups=[[0, 4], [1, 5], [2, 6], [3, 7]])

            if debug:
                for s in range(4):
                    nc.gpsimd.dma_start(dbg_qTh[s * 128:(s + 1) * 128, :],
                                        qTh[:, s, :])
                for s_ in range(4):
                    nc.gpsimd.dma_start(dbg_kTh[s_ * 128:(s_ + 1) * 128, :],
                                        kTh[:, s_, :])
                for kb in range(NKB):
                    nc.gpsimd.dma_start(
                        dbg_v[kb * 128:(kb + 1) * 128, :].rearrange(
                            "p (h d) -> p h d", h=8),
                        v_aug[:, kb, :, 0:HD])

            # ---------------- pair exchange of context ----------------
            _pst.close()

            if debug:
                nc.gpsimd.dma_start(dbg_ctx[:], ctx_own[:])
                for st in range(4):
                    for g in range(2):
                        nc.gpsimd.dma_start(
                            dbg_gath[g * GC + st * 128:g * GC + (st + 1) * 128, :],
                            ctx_gath[st, g * 128:(g + 1) * 128, :])

            # ---------------- output projection ----------------
            with (
                tc.tile_pool(name="wo", bufs=1) as wop,
                tc.tile_pool(name="pwo", bufs=4, space="PSUM") as pwo,
            ):
                wo_sb = wop.tile([128, NCH, HID], F32R)
                ctxg = wop.tile([128, NCH, QR], F32R)
                outT_sb = wop.tile([128, NCH, QR], F32)
                for hc in range(NCH):
                    hsl = slice(hc * 128, (hc + 1) * 128)
                    nc.gpsimd.dma_start(wo_sb[:, hc, :], w_o[hsl, :])
                    g, st = hc // 4, hc % 4
                    nc.gpsimd.dma_start(
                        ctxg[:, hc, :],
                        ctx_gath[st, g * 128:(g + 1) * 128, :])
                for oc in range(NCH):
                    for half in range(2):
                        ps = pwo.tile([128, 512], F32, tag="po")
                        hsl = slice(half * 512, (half + 1) * 512)
                        for hc in range(NCH):
                            nc.tensor.matmul(
                                ps[:],
                                _r32(wo_sb[:, hc, oc * 128:(oc + 1) * 128]),
                                _r32(ctxg[:, hc, hsl]),
                                start=(hc == 0), stop=(hc == NCH - 1))
                        nc.vector.tensor_copy(outT_sb[:, oc, hsl], ps[:])
                for oc in range(NCH):
                    nc.sync.dma_start(outT[oc * 128:(oc + 1) * 128, :],
                                      outT_sb[:, oc, :])

    nc.compile()
    return nc


def _get_nc():
    global _CACHED_NC
    if _CACHED_NC is None:
        _CACHED_NC = _build()
    return _CACHED_NC


def make_in_maps(query, key, value, w_q, w_k, w_v, w_o):
    qf = np.ascontiguousarray(query.reshape(B * Q, HID))
    ins = []
    for core in range(NCORE):
        g, r = core // 4, core % 4
        b = r // 2
        ins.append({
            "q_rows": np.ascontiguousarray(qf[r * QR:(r + 1) * QR]),
            "key_b": np.ascontiguousarray(key[b]),
            "value_b": np.ascontiguousarray(value[b]),
            "wq_s": np.ascontiguousarray(w_q[:, g * GC:(g + 1) * GC]),
            "wk_s": np.ascontiguousarray(w_k[:, g * GC:(g + 1) * GC]),
            "wv_s": np.ascontiguousarray(w_v[:, g * GC:(g + 1) * GC]),
            "w_o": np.ascontiguousarray(w_o),
        })
    return ins


def assemble(results):
    out = np.empty((B * Q, HID), np.float32)
    for r in range(4):
        out[r * QR:(r + 1) * QR, :] = results[r]["outT"].T
    return out.reshape(B, Q, HID)


_EXEC = None


def _get_exec():
    """Build the 8-core shard_map executable once; reuse across calls."""
    global _EXEC
    if _EXEC is not None:
        return _EXEC
    import jax
    from jax.sharding import Mesh, PartitionSpec
    from jax.experimental.shard_map import shard_map
    from concourse.bass2jax import (_bass_exec_p, install_neuronx_cc_hook,
                                    partition_id_tensor)

    install_neuronx_cc_hook()
    nc = _get_nc()
    in_names, out_names, out_avals, zero_outs = [], [], [], []
    for alloc in nc.m.functions[0].allocations:
        if not isinstance(alloc, mybir.MemoryLocationSet):
            continue
        name = alloc.memorylocations[0].name
        if alloc.kind == "ExternalInput":
            if name != "partition_id":
                in_names.append(name)
        elif alloc.kind == "ExternalOutput":
            out_names.append(name)
            shape = tuple(alloc.tensor_shape)
            dtype = mybir.dt.np(alloc.dtype)
            out_avals.append(jax.core.ShapedArray(shape, dtype))
            zero_outs.append(np.zeros(shape, dtype))
    partition_name = (nc.partition_id_tensor.name
                      if nc.partition_id_tensor else None)
    all_in = list(in_names) + list(out_names)
    if partition_name:
        all_in.append(partition_name)

    def _body(*args):
        operands = list(args)
        if partition_name is not None:
            operands.append(partition_id_tensor())
        return tuple(_bass_exec_p.bind(
            *operands, out_avals=tuple(out_avals), in_names=tuple(all_in),
            out_names=tuple(out_names), lowering_input_output_aliases=(),
            sim_require_finite=True, sim_require_nnan=True, nc=nc))

    devices = jax.devices()[:NCORE]
    mesh = Mesh(np.asarray(devices), ("core",))
    n_all = len(in_names) + len(out_names)
    fn = jax.jit(shard_map(_body, mesh=mesh,
                           in_specs=(PartitionSpec("core"),) * n_all,
                           out_specs=(PartitionSpec("core"),) * len(out_names),
                           check_rep=False), keep_unused=True)
    concat_zeros = [np.zeros((NCORE * z.shape[0], *z.shape[1:]), z.dtype)
                    for z in zero_outs]
    _EXEC = (fn, in_names, out_names, out_avals, concat_zeros)
    return _EXEC


def kernel(query, key, value, w_q, w_k, w_v, w_o):
    query = np.asarray(query, dtype=np.float32)
    key = np.asarray(key, dtype=np.float32)
    value = np.asarray(value, dtype=np.float32)
    ins = make_in_maps(query, key, value, np.asarray(w_q, np.float32),
                       np.asarray(w_k, np.float32), np.asarray(w_v, np.float32),
                       np.asarray(w_o, np.float32))
    fn, in_names, out_names, out_avals, concat_zeros = _get_exec()
    concat_in = [np.concatenate([np.asarray(ins[c][nm]) for c in range(NCORE)])
                 for nm in in_names]
    out_arrs = fn(*concat_in, *concat_zeros)
    results = [
        {nm: np.asarray(out_arrs[i]).reshape(NCORE, *out_avals[i].shape)[c]
         for i, nm in enumerate(out_names)}
        for c in range(NCORE)]
    return assemble(results)


if __name__ == "__main__":
    np.random.seed(0)
    q = np.random.randn(B, Q, HID).astype(np.float32)
    k = np.random.randn(B, KL, HID).astype(np.float32)
    v = np.random.randn(B, KL, HID).astype(np.float32)
    s = 1.0 / np.sqrt(HID)
    wq = (np.random.randn(HID, HID) * s).astype(np.float32)
    wk = (np.random.randn(HID, HID) * s).astype(np.float32)
    wv = (np.random.randn(HID, HID) * s).astype(np.float32)
    wo = (np.random.randn(HID, HID) * s).astype(np.float32)
    t0 = time.time()
    out = kernel(q, k, v, wq, wk, wv, wo)
    print("kernel done", time.time() - t0, out.shape)



# revision 45
# speedup vs baseline: 1.2895x; 1.2895x over previous
"""Trainium2 Bass kernel for MemoryEfficientCrossAttention (seq-parallel v2).

Problem (hardcoded): B=2, Q=2048, K=4096, HIDDEN=1024, HEADS=16, HEAD_DIM=64.
  out = softmax((x_q W_q)(x_k W_k)^T / sqrt(64)) (x_v W_v) W_o

Sharding over 8 NeuronCores: core = b*4 + kq
  b in {0,1}: batch;  kq in {0..3}: K-quarter (1024 contiguous k rows)
Each core projects q for ALL 2048 rows of its batch (4x redundant but cheap),
k/v for its local 1024 k rows (no redundancy), computes unnormalized
attention ctx^T (with a ones-column for the softmax denominators) against its
local k, then a 4-core ReduceScatter per 512-q block sums the partial ctx and
scatters 128-q fragments; each core normalizes its fragments and applies W_o
for its 4x128 output rows.  No other collectives.

Precision: activations and weights are cast to bf16 for the QKV projections
and attention matmuls (psum accumulation stays f32); the xbar DMA-transpose of
x runs on the single bf16 plane.  W_o path stays f32r.  Expected rel err
~5e-3 versus the f32 reference (gate is 2e-2).
"""

import os
import sys
import time

import numpy as np

sys.path.insert(0, "/opt/trn_rl_repo")

import concourse.bass as bass  # noqa: E402
import concourse.mybir as mybir  # noqa: E402
import concourse.tile as tile  # noqa: E402
from concourse import bacc  # noqa: E402

F32 = mybir.dt.float32
F32R = mybir.dt.float32r
BF16 = mybir.dt.bfloat16

HID = 1024
HEADS = 16
HD = 64
B = 2
Q = 2048
KL = 4096
NCORE = 8
KLOC = KL // 4        # local k rows per core
NCH = HID // 128      # 8 hidden chunks
CH_RT = 2             # 128-row tiles per transpose chunk
CHR = CH_RT * 128     # 256 rows per chunk
NKB = KLOC // 128     # 8 local k-blocks
NQB = 4               # 512-q blocks
QB = Q // NQB         # 512
GKB = 2               # k-blocks per score/exp group
SCALE = HD ** -0.5

_CACHED_NC = None


def _build():
    nc = bacc.Bacc("TRN2", target_bir_lowering=False, debug=False,
                   num_devices=NCORE)

    x_q = nc.dram_tensor("x_q", [Q, HID], F32, kind="ExternalInput")
    x_k = nc.dram_tensor("x_k", [KLOC, HID], F32, kind="ExternalInput")
    x_v = nc.dram_tensor("x_v", [KLOC, HID], F32, kind="ExternalInput")
    w_q = nc.dram_tensor("w_q", [HID, HID], F32, kind="ExternalInput")
    w_k = nc.dram_tensor("w_k", [HID, HID], F32, kind="ExternalInput")
    w_v = nc.dram_tensor("w_v", [HID, HID], F32, kind="ExternalInput")
    w_o = nc.dram_tensor("w_o", [HID, HID], F32, kind="ExternalInput")
    out_frag = nc.dram_tensor("out_frag", [NQB, 128, HID], F32,
                              kind="ExternalOutput")

    from contextlib import ExitStack

    with tile.TileContext(nc, pool_alloc_mode="queue") as tc:
        with tc.tile_pool(name="dram", bufs=1, space="DRAM") as dram:
            est = ExitStack()
            pp = est.enter_context(tc.tile_pool(name="persist", bufs=1))
            # partial ctx^T per 512-q block:
            # [head-half, qquad, head%8, d(+denom), 128 q]
            ctx_part = [dram.tile([2, 4, 8, HD + 1, 128], F32,
                                  name=f"ctx_part{i}")
                        for i in range(NQB)]
            frag = [dram.tile([HEADS, HD + 1, 128], F32, name=f"frag{i}")
                    for i in range(NQB)]

            qTh = pp.tile([128, NCH, Q], BF16)      # strip s = heads 2s,2s+1
            kTh = pp.tile([128, NCH, KLOC], BF16)
            v_aug = pp.tile([128, NKB, HEADS, HD + 1], BF16)

            ones = pp.tile([128, NKB * HEADS], BF16, name="ones")
            nc.vector.memset(ones[:], 1.0)
            nc.vector.tensor_copy(
                v_aug[:, :, :, HD],
                ones[:].rearrange("p (a b) -> p a b", a=NKB))

            # ---------------- staging + projections ----------------
            proj_st = ExitStack()
            xs = proj_st.enter_context(tc.tile_pool(name="xstage", bufs=2))
            xb = proj_st.enter_context(tc.tile_pool(name="xbf", bufs=3))
            xbt = proj_st.enter_context(tc.tile_pool(name="xT", bufs=4))
            ws = proj_st.enter_context(tc.tile_pool(name="wstage", bufs=1))
            wb = proj_st.enter_context(tc.tile_pool(name="wbf", bufs=8))
            pj = proj_st.enter_context(
                tc.tile_pool(name="pproj", bufs=2, space="PSUM"))

            def stage_chunk(src, row0, eng=None):
                """bf16-transposed [128, NCH, CHR] chunk of src rows.

                Loads go on SP (or gpsimd for the late wave); transposes go on
                the Activation queue so SP's in-order queue never serializes
                load(n+1) behind transpose(n)'s wait on the DVE cast.
                """
                eng = eng or nc.sync
                xf = xs.tile([128, CH_RT, HID], F32, tag="xf")
                eng.dma_start(
                    xf[:],
                    src[row0:row0 + CHR, :].rearrange("(t p) c -> p t c",
                                                      p=128))
                xc = xb.tile([128, CH_RT, HID], BF16, tag="xc")
                nc.vector.tensor_copy(xc[:], xf[:])
                # single xbar transpose of the flattened [128, 2048] block:
                # out[:, t, hc, r] = rows row0+t*128+r of hidden chunk hc
                xT = xbt.tile([128, CH_RT, NCH, 128], BF16, tag="xT")
                nc.scalar.dma_start_transpose(
                    xT[:].rearrange("p t h r -> p (t h) r"),
                    xc[:].rearrange("p t c -> p (t c)"))
                return xT

            def load_w_bf16(wdram, stage=None):
                """Load+cast w in quarters, optionally interleaving the
                staging of x chunks between quarters so the DMA device
                alternates weight and activation traffic at startup."""
                quarters = []
                for qq in range(4):
                    wf = ws.tile([128, 2, HID], F32, tag="wf")
                    nc.gpsimd.dma_start(
                        wf[:],
                        wdram[qq * CHR:(qq + 1) * CHR, :].rearrange(
                            "(t p) c -> p t c", p=128))
                    wh = wb.tile([128, 2, HID], BF16, tag="wb")
                    nc.scalar.copy(wh[:], wf[:])
                    quarters.append(wh)
                    if stage is not None:
                        stage(qq)
                return quarters

            def wsl(quarters, hc, csl):
                return quarters[hc // 2][:, hc % 2, csl]

            # K projection -> kTh (transposed strips), no redundancy.
            k_xT = []
            wk_h = load_w_bf16(
                w_k, stage=lambda c: k_xT.append(stage_chunk(x_k, c * CHR)))
            for c in range(KLOC // CHR):
                for s in range(NCH):
                    ps = pj.tile([128, CHR], F32, tag="ps")
                    for hc in range(NCH):
                        nc.tensor.matmul(
                            ps[:],
                            wsl(wk_h, hc, slice(s * 128, (s + 1) * 128)),
                            k_xT[c][:, :, hc, :],
                            start=(hc == 0), stop=(hc == NCH - 1))
                    nc.vector.tensor_copy(
                        kTh[:, s, c * CHR:(c + 1) * CHR], ps[:])

            # V projection -> v_aug (natural layout), no redundancy.
            v_xT = []
            wv_h = load_w_bf16(
                w_v, stage=lambda c: v_xT.append(stage_chunk(x_v, c * CHR)))
            for c in range(KLOC // CHR):
                for t in range(CH_RT):
                    kb = c * CH_RT + t
                    for sg in range(4):  # 256-col groups = 4 heads each
                        ps = pj.tile([128, CHR], F32, tag="ps")
                        for hc in range(NCH):
                            nc.tensor.matmul(
                                ps[:],
                                v_xT[c][:, t, hc, :],
                                wsl(wv_h, hc, slice(sg * 256, (sg + 1) * 256)),
                                start=(hc == 0), stop=(hc == NCH - 1))
                        nc.vector.tensor_copy(
                            v_aug[:, kb, 4 * sg:4 * sg + 4, 0:HD],
                            ps[:].rearrange("p (h d) -> p h d", h=4))

            # Q staging wave 1 (chunks 0-3, SP queue); wave 2 (4-7) issues on
            # the gpsimd queue mid-attention so its DMAs don't sit behind the
            # attention ctx stores on SP.
            q_xT = []
            wq_h = load_w_bf16(
                w_q, stage=lambda c: q_xT.append(stage_chunk(x_q, c * CHR)))

            def qproj_chunk(c):
                for s in range(NCH):
                    ps = pj.tile([128, CHR], F32, tag="ps")
                    for hc in range(NCH):
                        nc.tensor.matmul(
                            ps[:],
                            wsl(wq_h, hc, slice(s * 128, (s + 1) * 128)),
                            q_xT[c][:, :, hc, :],
                            start=(hc == 0), stop=(hc == NCH - 1))
                    nc.vector.tensor_copy(
                        qTh[:, s, c * CHR:(c + 1) * CHR], ps[:])

            # ---------------- attention + RS + output ----------------
            att_st = ExitStack()
            stp = att_st.enter_context(
                tc.tile_pool(name="pscore", bufs=2, space="PSUM"))
            ctxp = att_st.enter_context(
                tc.tile_pool(name="pctx", bufs=2, space="PSUM"))
            apool = att_st.enter_context(tc.tile_pool(name="apool", bufs=3))
            csb = att_st.enter_context(tc.tile_pool(name="ctxsb", bufs=2))
            wop = att_st.enter_context(tc.tile_pool(name="wo", bufs=1))
            fp = att_st.enter_context(tc.tile_pool(name="fragp", bufs=1))
            rp = att_st.enter_context(tc.tile_pool(name="normp", bufs=1))
            cnp = att_st.enter_context(tc.tile_pool(name="ctxNp", bufs=4))

            def qproj_strip(c, s):
                ps = pj.tile([128, CHR], F32, tag="ps")
                for hc in range(NCH):
                    nc.tensor.matmul(
                        ps[:],
                        wsl(wq_h, hc, slice(s * 128, (s + 1) * 128)),
                        q_xT[c][:, :, hc, :],
                        start=(hc == 0), stop=(hc == NCH - 1))
                nc.vector.tensor_copy(
                    qTh[:, s, c * CHR:(c + 1) * CHR], ps[:])

            def attn_qb(qb, next_chunks=None):
                qsl = slice(qb * QB, (qb + 1) * QB)
                for h in range(HEADS):
                    # Interleave one q-proj strip for the next q-block per
                    # head: keeps Act's exp stream dense across qb boundaries
                    # by filling PE's slack instead of a serial proj block.
                    if next_chunks is not None:
                        nc_c = next_chunks[h // NCH]
                        qproj_strip(nc_c, h % NCH)
                    s, i = h // 2, h % 2
                    psl = slice(i * HD, (i + 1) * HD)
                    ctx = ctxp.tile([HD + 1, QB], F32, tag="ctx",
                                    name=f"ctx_{qb}_{h}")
                    sts = []
                    As = []

                    def scores(g):
                        st = stp.tile([128, GKB, QB], F32, tag="st",
                                      name=f"st_{qb}_{h}_{g}")
                        for j in range(GKB):
                            kb = g * GKB + j
                            nc.tensor.matmul(
                                st[:, j, :],
                                kTh[psl, s, kb * 128:(kb + 1) * 128],
                                qTh[psl, s, qsl],
                                start=True, stop=True)
                        sts.append(st)
                        a = apool.tile([128, GKB, QB], BF16, tag="a",
                                       name=f"a_{qb}_{h}_{g}")
                        nc.scalar.activation(
                            a[:], st[:], mybir.ActivationFunctionType.Exp,
                            scale=SCALE)
                        As.append(a)

                    def pv(g):
                        for j in range(GKB):
                            kb = g * GKB + j
                            nc.tensor.matmul(
                                ctx[:],
                                v_aug[:, kb, h, :],
                                As[g][:, j, :],
                                start=(kb == 0), stop=(kb == NKB - 1))

                    ngr = NKB // GKB
                    scores(0)
                    scores(1)
                    for g in range(ngr):
                        if g + 2 < ngr:
                            scores(g + 2)
                        pv(g)
                    ctx_sb = csb.tile([HD + 1, QB], F32, tag="ctx_sb",
                                      name=f"ctxsb_{qb}_{h}")
                    nc.vector.tensor_copy(ctx_sb[:], ctx[:])
                    nc.sync.dma_start(
                        ctx_part[qb][h // 8, :, h % 8, :, :].rearrange(
                            "a p b -> p a b"),
                        ctx_sb[:].rearrange("p (a b) -> p a b", a=4))
                    if h == 7:
                        rs_qb(qb, 0)
                rs_qb(qb, 1)

            def rs_qb(qb, hf):
                nc.gpsimd.collective_compute(
                    "ReduceScatter", mybir.AluOpType.add,
                    ins=[ctx_part[qb][hf]],
                    outs=[frag[qb][hf * 8:(hf + 1) * 8]],
                    replica_groups=[[0, 1, 2, 3], [4, 5, 6, 7]])

            state = {}

            def load_wo():
                wo_sb = wop.tile([128, NCH, HID], BF16)
                for qq in range(4):
                    wf = ws.tile([128, 2, HID], F32, tag="wf")
                    for j in range(2):
                        hc = qq * 2 + j
                        nc.gpsimd.dma_start(wf[:, j, :],
                                            w_o[hc * 128:(hc + 1) * 128, :])
                    nc.vector.tensor_copy(
                        wo_sb[:, qq * 2:qq * 2 + 2, :], wf[:])
                state["wo_sb"] = wo_sb

            ctxNs = {}

            def norm_qb(qb):
                fsb = fp.tile([HD + 1, HEADS, 128], F32, tag="fsb")
                nc.gpsimd.dma_start(
                    fsb[:], frag[qb][:].rearrange("h p b -> p h b"))
                ctxN = cnp.tile([128, NCH, 128], BF16, tag="ctxN",
                                name=f"ctxN_{qb}")
                for hf in range(2):  # 8-head halves
                    hsl = slice(hf * 8, (hf + 1) * 8)
                    rinv = rp.tile([1, 8, 128], F32, tag="rinv",
                                   name=f"rinv_{qb}_{hf}")
                    nc.vector.reciprocal(rinv[:], fsb[HD:HD + 1, hsl, :])
                    rb = rp.tile([HD, 8, 128], F32, tag="rb",
                                 name=f"rb_{qb}_{hf}")
                    nc.gpsimd.partition_broadcast(rb[:], rinv[:])
                    fsb_r = fsb[0:HD, hsl, :].rearrange(
                        "p (c i) b -> p c i b", i=2)
                    rb_r = rb[:].rearrange("p (c i) b -> p c i b", i=2)
                    for i in range(2):
                        nc.vector.tensor_mul(
                            ctxN[i * HD:(i + 1) * HD, hf * 4:(hf + 1) * 4, :],
                            fsb_r[:, :, i, :],
                            rb_r[:, :, i, :])
                ctxNs[qb] = ctxN

            def out_qb(qb):
                wo_sb = state["wo_sb"]
                ctxN = ctxNs[qb]
                po_sb = fp.tile([128, HID], F32, tag="po_sb")
                po = stp.tile([128, GKB, QB], F32, tag="st",
                              name=f"po_{qb}")
                for half in range(2):
                    osl = slice(half * QB, (half + 1) * QB)
                    for hc in range(NCH):
                        nc.tensor.matmul(po[:, half, :], ctxN[:, hc, :],
                                         wo_sb[:, hc, osl],
                                         start=(hc == 0), stop=(hc == NCH - 1))
                nc.vector.tensor_copy(
                    po_sb[:], po[:].rearrange("p a b -> p (a b)"))
                nc.sync.dma_start(out_frag[qb], po_sb[:])

            # schedule: q-proj strips for block qb+1 are interleaved inside
            # attention block qb (one strip per head); RS issues per head-half.
            qproj_chunk(0)
            qproj_chunk(1)
            attn_qb(0, next_chunks=(2, 3))
            for c in range(4, 8):
                q_xT.append(stage_chunk(x_q, c * CHR, eng=nc.gpsimd))
            load_wo()
            attn_qb(1, next_chunks=(4, 5))
            norm_qb(0)
            attn_qb(2, next_chunks=(6, 7))
            norm_qb(1)
            attn_qb(3)
            norm_qb(2)
            out_qb(0)
            out_qb(1)
            out_qb(2)
            norm_qb(3)
            out_qb(3)
            att_st.close()
            proj_st.close()
            est.close()

    nc.compile()
    return nc


def _get_nc():
    global _CACHED_NC
    if _CACHED_NC is None:
        _CACHED_NC = _build()
    return _CACHED_NC


def make_in_maps(query, key, value, w_q, w_k, w_v, w_o):
    ins = []
    for core in range(NCORE):
        b, kq = core // 4, core % 4
        ins.append({
            "x_q": np.ascontiguousarray(query[b]),
            "x_k": np.ascontiguousarray(key[b][kq * KLOC:(kq + 1) * KLOC]),
            "x_v": np.ascontiguousarray(value[b][kq * KLOC:(kq + 1) * KLOC]),
            "w_q": np.ascontiguousarray(w_q),
            "w_k": np.ascontiguousarray(w_k),
            "w_v": np.ascontiguousarray(w_v),
            "w_o": np.ascontiguousarray(w_o),
        })
    return ins


def assemble(results):
    out = np.empty((B, Q, HID), np.float32)
    for core in range(NCORE):
        b, kq = core // 4, core % 4
        fragr = results[core]["out_frag"]
        for qb in range(NQB):
            r0 = qb * QB + kq * 128
            out[b, r0:r0 + 128, :] = fragr[qb]
    return out


_EXEC = None


def _get_exec():
    """Build the 8-core shard_map executable once; reuse across calls."""
    global _EXEC
    if _EXEC is not None:
        return _EXEC
    import jax
    from jax.sharding import Mesh, PartitionSpec
    from jax.experimental.shard_map import shard_map
    from concourse.bass2jax import (_bass_exec_p, install_neuronx_cc_hook,
                                    partition_id_tensor)

    install_neuronx_cc_hook()
    nc = _get_nc()
    in_names, out_names, out_avals, zero_outs = [], [], [], []
    for alloc in nc.m.functions[0].allocations:
        if not isinstance(alloc, mybir.MemoryLocationSet):
            continue
        name = alloc.memorylocations[0].name
        if alloc.kind == "ExternalInput":
            if name != "partition_id":
                in_names.append(name)
        elif alloc.kind == "ExternalOutput":
            out_names.append(name)
            shape = tuple(alloc.tensor_shape)
            dtype = mybir.dt.np(alloc.dtype)
            out_avals.append(jax.core.ShapedArray(shape, dtype))
            zero_outs.append(np.zeros(shape, dtype))
    partition_name = (nc.partition_id_tensor.name
                      if nc.partition_id_tensor else None)
    all_in = list(in_names) + list(out_names)
    if partition_name:
        all_in.append(partition_name)

    def _body(*args):
        operands = list(args)
        if partition_name is not None:
            operands.append(partition_id_tensor())
        return tuple(_bass_exec_p.bind(
            *operands, out_avals=tuple(out_avals), in_names=tuple(all_in),
            out_names=tuple(out_names), lowering_input_output_aliases=(),
            sim_require_finite=True, sim_require_nnan=True, nc=nc))

    devices = jax.devices()[:NCORE]
    mesh = Mesh(np.asarray(devices), ("core",))
    n_all = len(in_names) + len(out_names)
    fn = jax.jit(shard_map(_body, mesh=mesh,
                           in_specs=(PartitionSpec("core"),) * n_all,
                           out_specs=(PartitionSpec("core"),) * len(out_names),
                           check_rep=False), keep_unused=True)
    concat_zeros = [np.zeros((NCORE * z.shape[0], *z.shape[1:]), z.dtype)
                    for z in zero_outs]
    _EXEC = (fn, in_names, out_names, out_avals, concat_zeros)
    return _EXEC


def kernel(query, key, value, w_q, w_k, w_v, w_o):
    query = np.asarray(query, dtype=np.float32)
    key = np.asarray(key, dtype=np.float32)
    value = np.asarray(value, dtype=np.float32)
    ins = make_in_maps(query, key, value, np.asarray(w_q, np.float32),
                       np.asarray(w_k, np.float32), np.asarray(w_v, np.float32),
                       np.asarray(w_o, np.float32))
    fn, in_names, out_names, out_avals, concat_zeros = _get_exec()
    concat_in = [np.concatenate([np.asarray(ins[c][nm]) for c in range(NCORE)])
                 for nm in in_names]
    out_arrs = fn(*concat_in, *concat_zeros)
    results = [
        {nm: np.asarray(out_arrs[i]).reshape(NCORE, *out_avals[i].shape)[c]
         for i, nm in enumerate(out_names)}
        for c in range(NCORE)]
    return assemble(results)


if __name__ == "__main__":
    np.random.seed(0)
    q = np.random.randn(B, Q, HID).astype(np.float32)
    k = np.random.randn(B, KL, HID).astype(np.float32)
    v = np.random.randn(B, KL, HID).astype(np.float32)
    s = 1.0 / np.sqrt(HID)
    wq = (np.random.randn(HID, HID) * s).astype(np.float32)
    wk = (np.random.randn(HID, HID) * s).astype(np.float32)
    wv = (np.random.randn(HID, HID) * s).astype(np.float32)
    wo = (np.random.randn(HID, HID) * s).astype(np.float32)
    t0 = time.time()
    out = kernel(q, k, v, wq, wk, wv, wo)
    print("kernel done", time.time() - t0, out.shape)


# revision 54
# speedup vs baseline: 1.7653x; 1.3690x over previous
"""Trainium2 Bass kernel for MemoryEfficientCrossAttention (seq-parallel v2).

Problem (hardcoded): B=2, Q=2048, K=4096, HIDDEN=1024, HEADS=16, HEAD_DIM=64.
  out = softmax((x_q W_q)(x_k W_k)^T / sqrt(64)) (x_v W_v) W_o

Sharding over 8 NeuronCores: core = b*4 + kq
  b in {0,1}: batch;  kq in {0..3}: K-quarter (1024 contiguous k rows)
Each core projects q for ALL 2048 rows of its batch (4x redundant but cheap),
k/v for its local 1024 k rows (no redundancy), computes unnormalized
attention ctx^T (with a ones-column for the softmax denominators) against its
local k, then a 4-core ReduceScatter per 512-q block sums the partial ctx and
scatters 128-q fragments; each core normalizes its fragments and applies W_o
for its 4x128 output rows.  No other collectives.

Precision: activations and weights are cast to bf16 for the QKV projections
and attention matmuls (psum accumulation stays f32); the xbar DMA-transpose of
x runs on the single bf16 plane.  W_o path stays f32r.  Expected rel err
~5e-3 versus the f32 reference (gate is 2e-2).
"""

import os
import sys
import time

import numpy as np

sys.path.insert(0, "/opt/trn_rl_repo")

import concourse.bass as bass  # noqa: E402
import concourse.mybir as mybir  # noqa: E402
import concourse.tile as tile  # noqa: E402
from concourse import bacc  # noqa: E402

F32 = mybir.dt.float32
F32R = mybir.dt.float32r
BF16 = mybir.dt.bfloat16

HID = 1024
HEADS = 16
HD = 64
B = 2
Q = 2048
KL = 4096
NCORE = 8
KLOC = KL // 4        # local k rows per core
NCH = HID // 128      # 8 hidden chunks
CH_RT = 2             # 128-row tiles per transpose chunk
CHR = CH_RT * 128     # 256 rows per chunk
NKB = KLOC // 128     # 8 local k-blocks
NQB = 4               # 512-q blocks
QB = Q // NQB         # 512
GKB = 2               # k-blocks per score/exp group
SCALE = HD ** -0.5

_CACHED_NC = None


def _build():
    nc = bacc.Bacc("TRN2", target_bir_lowering=False, debug=False,
                   num_devices=NCORE)

    x_q = nc.dram_tensor("x_q", [Q, HID], F32, kind="ExternalInput")
    x_k = nc.dram_tensor("x_k", [KLOC, HID], F32, kind="ExternalInput")
    x_v = nc.dram_tensor("x_v", [KLOC, HID], F32, kind="ExternalInput")
    w_q = nc.dram_tensor("w_q", [HID, HID], F32, kind="ExternalInput")
    w_k = nc.dram_tensor("w_k", [HID, HID], F32, kind="ExternalInput")
    w_v = nc.dram_tensor("w_v", [HID, HID], F32, kind="ExternalInput")
    w_o = nc.dram_tensor("w_o", [HID, HID], F32, kind="ExternalInput")
    out_frag = nc.dram_tensor("out_frag", [NQB, 128, HID], F32,
                              kind="ExternalOutput")

    from contextlib import ExitStack

    with tile.TileContext(nc, pool_alloc_mode="queue") as tc:
        with tc.tile_pool(name="dram", bufs=1, space="DRAM") as dram:
            est = ExitStack()
            pp = est.enter_context(tc.tile_pool(name="persist", bufs=1))
            # partial ctx^T per 512-q block:
            # [head-half, qquad, head%8, d(+denom), 128 q]
            shared = "Shared" if os.environ.get("KSHARED", "0") == "1" \
                else "Local"
            rs_mode = os.environ.get("KRS", "half")
            # seg_h heads per collective segment; each segment is its own
            # contiguous DRAM tensor (BIR requires contiguous collective in).
            seg_h = 8 if rs_mode == "half" else HEADS
            nseg = HEADS // seg_h
            ctx_part = [[dram.tile([4, seg_h, HD + 1, 128], BF16,
                                   name=f"ctx_part{i}_{s}")
                         for s in range(nseg)]
                        for i in range(NQB)]
            a2a_out = [dram.tile([4, HEADS, HD + 1, 128], BF16,
                                 name=f"a2a_out{i}")
                       for i in range(NQB)]
            frag = [dram.tile([HEADS, HD + 1, 128], BF16, name=f"frag{i}",
                              addr_space=shared)
                    for i in range(NQB)]

            qTh = pp.tile([128, NCH, Q], BF16)      # strip s = heads 2s,2s+1
            kTh = pp.tile([128, NCH, KLOC], BF16)
            v_aug = pp.tile([128, NKB, HEADS, HD + 1], BF16)

            ones = pp.tile([128, NKB * HEADS], BF16, name="ones")
            nc.vector.memset(ones[:], 1.0)
            nc.vector.tensor_copy(
                v_aug[:, :, :, HD],
                ones[:].rearrange("p (a b) -> p a b", a=NKB))

            # ---------------- staging + projections ----------------
            proj_st = ExitStack()
            xs = proj_st.enter_context(tc.tile_pool(name="xstage", bufs=2))
            xb = proj_st.enter_context(tc.tile_pool(name="xbf", bufs=3))
            xbt = proj_st.enter_context(tc.tile_pool(name="xT", bufs=4))
            ws = proj_st.enter_context(tc.tile_pool(name="wstage", bufs=1))
            wb = proj_st.enter_context(tc.tile_pool(name="wbf", bufs=8))
            pj = proj_st.enter_context(
                tc.tile_pool(name="pproj", bufs=2, space="PSUM"))

            def stage_chunk(src, row0, eng=None):
                """bf16-transposed [128, NCH, CHR] chunk of src rows.

                Loads go on SP (or gpsimd for the late wave); transposes go on
                the Activation queue so SP's in-order queue never serializes
                load(n+1) behind transpose(n)'s wait on the DVE cast.
                """
                eng = eng or nc.sync
                xf = xs.tile([128, CH_RT, HID], F32, tag="xf")
                eng.dma_start(
                    xf[:],
                    src[row0:row0 + CHR, :].rearrange("(t p) c -> p t c",
                                                      p=128))
                xc = xb.tile([128, CH_RT, HID], BF16, tag="xc")
                nc.vector.tensor_copy(xc[:], xf[:])
                # single xbar transpose of the flattened [128, 2048] block:
                # out[:, t, hc, r] = rows row0+t*128+r of hidden chunk hc
                xT = xbt.tile([128, CH_RT, NCH, 128], BF16, tag="xT")
                nc.scalar.dma_start_transpose(
                    xT[:].rearrange("p t h r -> p (t h) r"),
                    xc[:].rearrange("p t c -> p (t c)"))
                return xT

            def load_w_bf16(wdram, stage=None):
                """Load+cast w in quarters, optionally interleaving the
                staging of x chunks between quarters so the DMA device
                alternates weight and activation traffic at startup."""
                quarters = []
                for qq in range(4):
                    wf = ws.tile([128, 2, HID], F32, tag="wf")
                    nc.gpsimd.dma_start(
                        wf[:],
                        wdram[qq * CHR:(qq + 1) * CHR, :].rearrange(
                            "(t p) c -> p t c", p=128))
                    wh = wb.tile([128, 2, HID], BF16, tag="wb")
                    nc.scalar.copy(wh[:], wf[:])
                    quarters.append(wh)
                    if stage is not None:
                        stage(qq)
                return quarters

            def wsl(quarters, hc, csl):
                return quarters[hc // 2][:, hc % 2, csl]

            # K projection -> kTh (transposed strips), no redundancy.
            k_xT = []
            wk_h = load_w_bf16(
                w_k, stage=lambda c: k_xT.append(stage_chunk(x_k, c * CHR)))
            for c in range(KLOC // CHR):
                for s in range(NCH):
                    ps = pj.tile([128, CHR], F32, tag="ps")
                    for hc in range(NCH):
                        nc.tensor.matmul(
                            ps[:],
                            wsl(wk_h, hc, slice(s * 128, (s + 1) * 128)),
                            k_xT[c][:, :, hc, :],
                            start=(hc == 0), stop=(hc == NCH - 1))
                    nc.vector.tensor_copy(
                        kTh[:, s, c * CHR:(c + 1) * CHR], ps[:])

            # V projection -> v_aug (natural layout), no redundancy.
            v_xT = []
            wv_h = load_w_bf16(
                w_v, stage=lambda c: v_xT.append(stage_chunk(x_v, c * CHR)))
            for c in range(KLOC // CHR):
                for t in range(CH_RT):
                    kb = c * CH_RT + t
                    for sg in range(4):  # 256-col groups = 4 heads each
                        ps = pj.tile([128, CHR], F32, tag="ps")
                        for hc in range(NCH):
                            nc.tensor.matmul(
                                ps[:],
                                v_xT[c][:, t, hc, :],
                                wsl(wv_h, hc, slice(sg * 256, (sg + 1) * 256)),
                                start=(hc == 0), stop=(hc == NCH - 1))
                        nc.vector.tensor_copy(
                            v_aug[:, kb, 4 * sg:4 * sg + 4, 0:HD],
                            ps[:].rearrange("p (h d) -> p h d", h=4))

            # Q staging wave 1 (chunks 0-3, SP queue); wave 2 (4-7) issues on
            # the gpsimd queue mid-attention so its DMAs don't sit behind the
            # attention ctx stores on SP.
            q_xT = []
            wq_h = load_w_bf16(
                w_q, stage=lambda c: q_xT.append(stage_chunk(x_q, c * CHR)))

            def qproj_chunk(c):
                for s in range(NCH):
                    ps = pj.tile([128, CHR], F32, tag="ps")
                    for hc in range(NCH):
                        nc.tensor.matmul(
                            ps[:],
                            wsl(wq_h, hc, slice(s * 128, (s + 1) * 128)),
                            q_xT[c][:, :, hc, :],
                            start=(hc == 0), stop=(hc == NCH - 1))
                    nc.vector.tensor_copy(
                        qTh[:, s, c * CHR:(c + 1) * CHR], ps[:])

            # ---------------- attention + RS + output ----------------
            att_st = ExitStack()
            stp = att_st.enter_context(
                tc.tile_pool(name="pscore", bufs=2, space="PSUM"))
            ctxp = att_st.enter_context(
                tc.tile_pool(name="pctx", bufs=2, space="PSUM"))
            apool = att_st.enter_context(tc.tile_pool(name="apool", bufs=3))
            csb = att_st.enter_context(tc.tile_pool(name="ctxsb", bufs=2))
            wop = att_st.enter_context(tc.tile_pool(name="wo", bufs=1))
            fp = att_st.enter_context(tc.tile_pool(name="fragp", bufs=1))
            rp = att_st.enter_context(tc.tile_pool(name="normp", bufs=1))
            cnp = att_st.enter_context(tc.tile_pool(name="ctxNp", bufs=4))

            def qproj_strip(c, s):
                ps = pj.tile([128, CHR], F32, tag="ps")
                for hc in range(NCH):
                    nc.tensor.matmul(
                        ps[:],
                        wsl(wq_h, hc, slice(s * 128, (s + 1) * 128)),
                        q_xT[c][:, :, hc, :],
                        start=(hc == 0), stop=(hc == NCH - 1))
                nc.vector.tensor_copy(
                    qTh[:, s, c * CHR:(c + 1) * CHR], ps[:])

            def attn_qb(qb, next_chunks=None):
                qsl = slice(qb * QB, (qb + 1) * QB)
                for h in range(HEADS):
                    # Interleave one q-proj strip for the next q-block per
                    # head: keeps Act's exp stream dense across qb boundaries
                    # by filling PE's slack instead of a serial proj block.
                    if next_chunks is not None:
                        nc_c = next_chunks[h // NCH]
                        qproj_strip(nc_c, h % NCH)
                    s, i = h // 2, h % 2
                    psl = slice(i * HD, (i + 1) * HD)
                    ctx = ctxp.tile([HD + 1, QB], F32, tag="ctx",
                                    name=f"ctx_{qb}_{h}")
                    sts = []
                    As = []

                    def scores(g):
                        st = stp.tile([128, GKB, QB], F32, tag="st",
                                      name=f"st_{qb}_{h}_{g}")
                        for j in range(GKB):
                            kb = g * GKB + j
                            nc.tensor.matmul(
                                st[:, j, :],
                                kTh[psl, s, kb * 128:(kb + 1) * 128],
                                qTh[psl, s, qsl],
                                start=True, stop=True)
                        sts.append(st)
                        a = apool.tile([128, GKB, QB], BF16, tag="a",
                                       name=f"a_{qb}_{h}_{g}")
                        nc.scalar.activation(
                            a[:], st[:], mybir.ActivationFunctionType.Exp,
                            scale=SCALE)
                        As.append(a)

                    def pv(g):
                        for j in range(GKB):
                            kb = g * GKB + j
                            nc.tensor.matmul(
                                ctx[:],
                                v_aug[:, kb, h, :],
                                As[g][:, j, :],
                                start=(kb == 0), stop=(kb == NKB - 1))

                    ngr = NKB // GKB
                    scores(0)
                    scores(1)
                    for g in range(ngr):
                        if g + 2 < ngr:
                            scores(g + 2)
                        pv(g)
                    ctx_sb = csb.tile([HD + 1, QB], BF16, tag="ctx_sb",
                                      name=f"ctxsb_{qb}_{h}")
                    nc.vector.tensor_copy(ctx_sb[:], ctx[:])
                    nc.sync.dma_start(
                        ctx_part[qb][h // seg_h][:, h % seg_h, :, :].rearrange(
                            "a p b -> p a b"),
                        ctx_sb[:].rearrange("p (a b) -> p a b", a=4))
                    if (h + 1) % seg_h == 0:
                        rs_qb(qb, h // seg_h)

            RGROUPS = [[0, 1, 2, 3], [4, 5, 6, 7]]

            def rs_qb(qb, seg):
                if rs_mode == "none":
                    return  # timing-only floor: skip collectives
                if rs_mode == "a2a":
                    nc.gpsimd.collective_compute(
                        "AllToAll", mybir.AluOpType.bypass,
                        ins=[ctx_part[qb][seg][:]],
                        outs=[a2a_out[qb][:]],
                        replica_groups=RGROUPS)
                    return
                nc.gpsimd.collective_compute(
                    "ReduceScatter", mybir.AluOpType.add,
                    ins=[ctx_part[qb][seg][:]],
                    outs=[frag[qb][seg * seg_h:(seg + 1) * seg_h]],
                    replica_groups=RGROUPS)

            state = {}

            def load_wo():
                wo_sb = wop.tile([128, NCH, HID], BF16)
                for qq in range(4):
                    wf = ws.tile([128, 2, HID], F32, tag="wf")
                    for j in range(2):
                        hc = qq * 2 + j
                        nc.gpsimd.dma_start(wf[:, j, :],
                                            w_o[hc * 128:(hc + 1) * 128, :])
                    nc.vector.tensor_copy(
                        wo_sb[:, qq * 2:qq * 2 + 2, :], wf[:])
                state["wo_sb"] = wo_sb

            ctxNs = {}

            def norm_qb(qb):
                fsb = fp.tile([HD + 1, HEADS, 128], BF16, tag="fsb")
                if rs_mode == "a2a":
                    # sum the 4 source blocks routed to us by the AllToAll
                    fsc = fp.tile([HD + 1, HEADS, 128], BF16, tag="fsc")
                    nc.gpsimd.dma_start(
                        fsb[:], a2a_out[qb][0].rearrange("h p b -> p h b"))
                    for src in range(1, 4):
                        nc.gpsimd.dma_start(
                            fsc[:],
                            a2a_out[qb][src].rearrange("h p b -> p h b"))
                        nc.vector.tensor_add(fsb[:], fsb[:], fsc[:])
                else:
                    nc.gpsimd.dma_start(
                        fsb[:], frag[qb][:].rearrange("h p b -> p h b"))
                ctxN = cnp.tile([128, NCH, 128], BF16, tag="ctxN",
                                name=f"ctxN_{qb}")
                for hf in range(2):  # 8-head halves
                    hsl = slice(hf * 8, (hf + 1) * 8)
                    rinv = rp.tile([1, 8, 128], F32, tag="rinv",
                                   name=f"rinv_{qb}_{hf}")
                    nc.vector.reciprocal(rinv[:], fsb[HD:HD + 1, hsl, :])
                    rb = rp.tile([HD, 8, 128], F32, tag="rb",
                                 name=f"rb_{qb}_{hf}")
                    nc.gpsimd.partition_broadcast(rb[:], rinv[:])
                    fsb_r = fsb[0:HD, hsl, :].rearrange(
                        "p (c i) b -> p c i b", i=2)
                    rb_r = rb[:].rearrange("p (c i) b -> p c i b", i=2)
                    for i in range(2):
                        nc.vector.tensor_mul(
                            ctxN[i * HD:(i + 1) * HD, hf * 4:(hf + 1) * 4, :],
                            fsb_r[:, :, i, :],
                            rb_r[:, :, i, :])
                ctxNs[qb] = ctxN

            def out_qb(qb):
                wo_sb = state["wo_sb"]
                ctxN = ctxNs[qb]
                po_sb = fp.tile([128, HID], F32, tag="po_sb")
                po = stp.tile([128, GKB, QB], F32, tag="st",
                              name=f"po_{qb}")
                for half in range(2):
                    osl = slice(half * QB, (half + 1) * QB)
                    for hc in range(NCH):
                        nc.tensor.matmul(po[:, half, :], ctxN[:, hc, :],
                                         wo_sb[:, hc, osl],
                                         start=(hc == 0), stop=(hc == NCH - 1))
                nc.vector.tensor_copy(
                    po_sb[:], po[:].rearrange("p a b -> p (a b)"))
                nc.sync.dma_start(out_frag[qb], po_sb[:])

            # schedule: q-proj strips for block qb+1 are interleaved inside
            # attention block qb (one strip per head); RS issues per head-half.
            qproj_chunk(0)
            qproj_chunk(1)
            attn_qb(0, next_chunks=(2, 3))
            for c in range(4, 8):
                q_xT.append(stage_chunk(x_q, c * CHR, eng=nc.gpsimd))
            load_wo()
            attn_qb(1, next_chunks=(4, 5))
            norm_qb(0)
            attn_qb(2, next_chunks=(6, 7))
            norm_qb(1)
            attn_qb(3)
            norm_qb(2)
            out_qb(0)
            out_qb(1)
            out_qb(2)
            norm_qb(3)
            out_qb(3)
            att_st.close()
            proj_st.close()
            est.close()

    nc.compile()
    return nc


def _get_nc():
    global _CACHED_NC
    if _CACHED_NC is None:
        _CACHED_NC = _build()
    return _CACHED_NC


def make_in_maps(query, key, value, w_q, w_k, w_v, w_o):
    ins = []
    for core in range(NCORE):
        b, kq = core // 4, core % 4
        ins.append({
            "x_q": np.ascontiguousarray(query[b]),
            "x_k": np.ascontiguousarray(key[b][kq * KLOC:(kq + 1) * KLOC]),
            "x_v": np.ascontiguousarray(value[b][kq * KLOC:(kq + 1) * KLOC]),
            "w_q": np.ascontiguousarray(w_q),
            "w_k": np.ascontiguousarray(w_k),
            "w_v": np.ascontiguousarray(w_v),
            "w_o": np.ascontiguousarray(w_o),
        })
    return ins


def assemble(results):
    out = np.empty((B, Q, HID), np.float32)
    for core in range(NCORE):
        b, kq = core // 4, core % 4
        fragr = results[core]["out_frag"]
        for qb in range(NQB):
            r0 = qb * QB + kq * 128
            out[b, r0:r0 + 128, :] = fragr[qb]
    return out


_EXEC = None


def _get_exec():
    """Build the 8-core shard_map executable once; reuse across calls."""
    global _EXEC
    if _EXEC is not None:
        return _EXEC
    import jax
    from jax.sharding import Mesh, PartitionSpec
    from jax.experimental.shard_map import shard_map
    from concourse.bass2jax import (_bass_exec_p, install_neuronx_cc_hook,
                                    partition_id_tensor)

    install_neuronx_cc_hook()
    nc = _get_nc()
    in_names, out_names, out_avals, zero_outs = [], [], [], []
    for alloc in nc.m.functions[0].allocations:
        if not isinstance(alloc, mybir.MemoryLocationSet):
            continue
        name = alloc.memorylocations[0].name
        if alloc.kind == "ExternalInput":
            if name != "partition_id":
                in_names.append(name)
        elif alloc.kind == "ExternalOutput":
            out_names.append(name)
            shape = tuple(alloc.tensor_shape)
            dtype = mybir.dt.np(alloc.dtype)
            out_avals.append(jax.core.ShapedArray(shape, dtype))
            zero_outs.append(np.zeros(shape, dtype))
    partition_name = (nc.partition_id_tensor.name
                      if nc.partition_id_tensor else None)
    all_in = list(in_names) + list(out_names)
    if partition_name:
        all_in.append(partition_name)

    def _body(*args):
        operands = list(args)
        if partition_name is not None:
            operands.append(partition_id_tensor())
        return tuple(_bass_exec_p.bind(
            *operands, out_avals=tuple(out_avals), in_names=tuple(all_in),
            out_names=tuple(out_names), lowering_input_output_aliases=(),
            sim_require_finite=True, sim_require_nnan=True, nc=nc))

    devices = jax.devices()[:NCORE]
    mesh = Mesh(np.asarray(devices), ("core",))
    n_all = len(in_names) + len(out_names)
    fn = jax.jit(shard_map(_body, mesh=mesh,
                           in_specs=(PartitionSpec("core"),) * n_all,
                           out_specs=(PartitionSpec("core"),) * len(out_names),
                           check_rep=False), keep_unused=True)
    concat_zeros = [np.zeros((NCORE * z.shape[0], *z.shape[1:]), z.dtype)
                    for z in zero_outs]
    _EXEC = (fn, in_names, out_names, out_avals, concat_zeros)
    return _EXEC


def kernel(query, key, value, w_q, w_k, w_v, w_o):
    query = np.asarray(query, dtype=np.float32)
    key = np.asarray(key, dtype=np.float32)
    value = np.asarray(value, dtype=np.float32)
    ins = make_in_maps(query, key, value, np.asarray(w_q, np.float32),
                       np.asarray(w_k, np.float32), np.asarray(w_v, np.float32),
                       np.asarray(w_o, np.float32))
    fn, in_names, out_names, out_avals, concat_zeros = _get_exec()
    concat_in = [np.concatenate([np.asarray(ins[c][nm]) for c in range(NCORE)])
                 for nm in in_names]
    out_arrs = fn(*concat_in, *concat_zeros)
    results = [
        {nm: np.asarray(out_arrs[i]).reshape(NCORE, *out_avals[i].shape)[c]
         for i, nm in enumerate(out_names)}
        for c in range(NCORE)]
    return assemble(results)


if __name__ == "__main__":
    np.random.seed(0)
    q = np.random.randn(B, Q, HID).astype(np.float32)
    k = np.random.randn(B, KL, HID).astype(np.float32)
    v = np.random.randn(B, KL, HID).astype(np.float32)
    s = 1.0 / np.sqrt(HID)
    wq = (np.random.randn(HID, HID) * s).astype(np.float32)
    wk = (np.random.randn(HID, HID) * s).astype(np.float32)
    wv = (np.random.randn(HID, HID) * s).astype(np.float32)
    wo = (np.random.randn(HID, HID) * s).astype(np.float32)
    t0 = time.time()
    out = kernel(q, k, v, wq, wk, wv, wo)
    print("kernel done", time.time() - t0, out.shape)
